# revision 65
# baseline (speedup 1.0000x reference)
"""Multi-head causal attention (B=2, S=2048, D=1024, H=16) on 8 TRN2 cores.

Sharding: core = (batch, group-of-4-heads). Each core computes attention for
its 4 heads of its batch and a rank-256 partial of the output projection;
the host sums the 4 partials per batch. The interleaved head split of the
reference (head h = columns h::16) is undone on the host by permuting the
weight matrices, so on-chip everything is head-contiguous.

bf16 end-to-end (PSUM accumulation stays fp32), host-side repack of all
inputs into exact SBUF layouts (single-descriptor-per-partition DMAs), direct
V^T projection from x^T tiles (no PE transposes), diagonal score tiles
computed only over the live q range with a single shared 128x128 tril mask.

On-chip layout (per core):
  QT/KT [128, 2048] bf16  head-pair-stacked transposed Q/K (pair p, heads A/B
                          on partitions 0:64 / 64:128)
  S^T   [128, 2, 512]     scores for a (k-tile, q-chunk) in PSUM; the two
                          64-contraction matmuls run on separate PE row groups
  exp   one ScalarE activation per k-tile over both heads' live q range
  PV    oAB[65, 2, 512] += Vaug^T @ P^T; Vaug carries a ones column so row 64
                          accumulates the softmax denominator l
  norm  l row to SBUF (DVE), reciprocal (DVE), broadcast across 64 partitions
        on the otherwise-idle GpSimd engine (partition_broadcast from the
        `attn` ucode library), two fused PSUM-read multiplies into AT
  out   partial = A^T.T @ ow accumulated over the 2 pairs in PSUM, staged to
        bf16 and summed on the host across the 4 head-group cores

Scheduling: per chunk j the two pair-rounds run back-to-back; next-chunk
projection pieces feed the PE between score/PV matmuls so ScalarE exp
latency is hidden; all output-projection pieces are deferred into the
exp-bound last-chunk rounds; the last pair's normalization runs per
128-column quarter, each immediately feeding its output-projection piece.
"""
import sys
sys.path.insert(0, '/opt/trn_rl_repo')

import numpy as np

DIM = 1024
HEADS = 16
S = 2048
B = 2
HD = 64
N_CORES = 8
HPC = 4          # heads per core
PAIRS = 2        # processed as 2 pairs of heads (pair packs the 128-wide PE)
QCH = 512        # q chunk
NKT = S // 128   # k tiles per sequence

_nc_cache = None


def _build(debug=False):
    import concourse.bass as bass
    import concourse.tile as tile
    import concourse.mybir as mybir
    from concourse import bacc
    from concourse import library_config
    from contextlib import ExitStack

    f32 = mybir.dt.float32
    bf16 = mybir.dt.bfloat16
    Exp = mybir.ActivationFunctionType.Exp

    nc = bacc.Bacc("TRN2", target_bir_lowering=False, debug=False,
                   enable_asserts=False, num_devices=N_CORES)

    # all inputs host-repacked to SBUF layout (partition dim first, contiguous)
    xT = nc.dram_tensor("xT", [128, 32, QCH], bf16, kind="ExternalInput").ap()
    qw = nc.dram_tensor("qw", [128, 2, 8, 128], bf16, kind="ExternalInput").ap()
    kw = nc.dram_tensor("kw", [128, 2, 8, 128], bf16, kind="ExternalInput").ap()
    vw = nc.dram_tensor("vw", [128, 8, 256], bf16, kind="ExternalInput").ap()
    ow = nc.dram_tensor("ow", [128, 2, DIM], bf16, kind="ExternalInput").ap()
    mask = nc.dram_tensor("mask", [128, 128], bf16, kind="ExternalInput").ap()
    out = nc.dram_tensor("out", [S, DIM], bf16, kind="ExternalOutput").ap()

    with tile.TileContext(nc) as tc, ExitStack() as ctx:
        const_pool = ctx.enter_context(tc.tile_pool(name="const", bufs=1))
        xin_pool = ctx.enter_context(tc.tile_pool(name="xin", bufs=2))
        big_pool = ctx.enter_context(tc.tile_pool(name="big", bufs=1))
        pt_pool = ctx.enter_context(tc.tile_pool(name="pt", bufs=4))
        small_pool = ctx.enter_context(tc.tile_pool(name="small", bufs=4))
        outst_pool = ctx.enter_context(tc.tile_pool(name="outst", bufs=3))
        psum_s = ctx.enter_context(tc.tile_pool(name="psum_s", bufs=2, space="PSUM"))
        psum_o = ctx.enter_context(tc.tile_pool(name="psum_o", bufs=2, space="PSUM"))

        # constants; issue order matters: chunk-0 x first, then qw/kw (first
        # matmuls), mask (needed at j=0 diagonal), vw, then ow (needed last)
        xts = {}

        def xt_dma(j):
            xt = xin_pool.tile([128, 8, QCH], bf16, tag="xt", name="xt")
            nc.sync.dma_start(out=xt, in_=xT[:, j * 8:(j + 1) * 8, :])
            xts[j] = xt

        # qw first (lhsT of the first matmuls), then x chunk 0 in quarters
        # so Q-proj starts as soon as the first two d-tiles land
        qw_sb = const_pool.tile([128, 2, 8, 128], bf16, tag="qw")
        kw_sb = const_pool.tile([128, 2, 8, 128], bf16, tag="kw")
        vw_sb = const_pool.tile([128, 8, 256], bf16, tag="vw")
        xt0 = xin_pool.tile([128, 8, QCH], bf16, tag="xt", name="xt0")
        nc.sync.dma_start(out=xt0[:, 0:4, :], in_=xT[:, 0:4, :])
        nc.sync.dma_start(out=qw_sb[:, 0], in_=qw[:, 0])
        nc.sync.dma_start(out=xt0[:, 4:8, :], in_=xT[:, 4:8, :])
        nc.sync.dma_start(out=qw_sb[:, 1], in_=qw[:, 1])
        xts[0] = xt0
        nc.sync.dma_start(out=kw_sb[:, 0], in_=kw[:, 0])
        nc.sync.dma_start(out=kw_sb[:, 1], in_=kw[:, 1])
        mask_sb = const_pool.tile([128, 128], bf16, tag="mask")
        nc.gpsimd.dma_start(out=mask_sb, in_=mask)
        nc.gpsimd.dma_start(out=vw_sb, in_=vw)
        # partition_broadcast ucode library: issued after the gpsimd-queue
        # DMAs so it doesn't delay vw; first use is the first finalize
        nc.gpsimd.load_library(library_config.attn)
        ow_sb = const_pool.tile([128, 2, DIM], bf16, tag="ow")

        # PE p-state warmup: the clock needs ~3us of continuous execution to
        # ramp 1.2 -> 2.4 GHz, and the PE would otherwise idle ~6us waiting
        # for the first input DMAs. Dummy zero-matmuls (results never read)
        # keep it busy so the first real matmuls start at full clock.
        zw = const_pool.tile([128, 128], bf16, tag="zw")
        nc.vector.memset(zw, 0.0)
        zx = const_pool.tile([128, QCH], bf16, tag="zx")
        nc.vector.memset(zx, 0.0)
        for _ in range(14):
            wps = psum_s.tile([128, QCH], f32, tag="s2", name="wps")
            nc.tensor.matmul(wps, zw, zx, start=True, stop=True)

        QT = [big_pool.tile([128, S], bf16, tag=f"QT{p}", name=f"QT{p}")
              for p in range(PAIRS)]
        KT = [big_pool.tile([128, S], bf16, tag=f"KT{p}", name=f"KT{p}")
              for p in range(PAIRS)]
        AT = [big_pool.tile([128, S], bf16, tag=f"AT{p}", name=f"AT{p}")
              for p in range(PAIRS)]
        # V in cols 0:64, ones in col 64 (denominator accumulates on oAB
        # partition 64 -- DVE operand partition bases must be 0/32/64/96)
        Vaug = big_pool.tile([128, NKT, HPC, HD + 1], bf16, tag="Vaug",
                             name="Vaug")
        nc.vector.memset(Vaug[:, :, :, HD:HD + 1], 1.0)

        # ---- Projection pieces (fed into the attention loop so ScalarE exp
        # ---- work overlaps projection PE work) ----
        def proj_piece(wsb, dstT, j, ct):
            qsl = slice(j * QCH, (j + 1) * QCH)
            ps1 = psum_s.tile([128, QCH], f32, tag="s2", name="ps1")
            for kt in range(8):
                nc.tensor.matmul(
                    ps1,
                    wsb[:, ct, kt, :],
                    xts[j][:, kt, :],
                    start=(kt == 0), stop=(kt == 7))
            nc.vector.tensor_copy(out=dstT[ct][:, qsl], in_=ps1)

        def vproj_piece(j, ktl):
            # V^T [k-tile 128, 4 heads x 64] directly from x^T tiles; the
            # x^T tile is stationary so the 256-wide vw stream hides loads
            ktg = 4 * j + ktl
            pv = psum_s.tile([128, 256], f32, tag="s2", name="pv")
            for kt in range(8):
                nc.tensor.matmul(
                    pv,
                    xts[j][:, kt, ktl * 128:(ktl + 1) * 128],
                    vw_sb[:, kt, :],
                    start=(kt == 0), stop=(kt == 7))
            nc.vector.tensor_copy(
                out=Vaug[:, ktg, :, 0:HD],
                in_=pv.rearrange("p (h d) -> p h d", h=HPC))

        def round_feed(j):
            # work pieces producing round-j inputs, sprinkled into round j-1
            items = [lambda: xt_dma(j)] if j > 0 else []
            if j == 1:
                # ow is only needed by the output projection (last rounds);
                # issuing it here keeps it off the startup queue's long pole
                items.append(lambda: nc.sync.dma_start(out=ow_sb, in_=ow))
            for wsb, dstT in ((qw_sb, QT), (kw_sb, KT)):
                for ct in range(PAIRS):
                    items.append(
                        lambda w=wsb, d=dstT, c=ct: proj_piece(w, d, j, c))
            for ktl in range(4):
                items.append(lambda k=ktl: vproj_piece(j, k))
            return items

        # prologue: round 0 inputs emitted directly
        for piece in round_feed(0):
            piece()

        # ---- Phase 2: causal attention, pair-packed; then output proj ----
        def finalize_norm(p, j, oAB, c0=0, c1=QCH):
            # l row (oAB partition 64) to SBUF, reciprocal on DVE,
            # broadcast across 64 partitions on the idle GpSimd engine, two
            # fused PSUM-read multiplies into AT. No PE/ScalarE-blocking ops.
            w = c1 - c0
            qsl = slice(j * QCH + c0, j * QCH + c1)
            csl = slice(c0, c1)
            lsb = small_pool.tile([1, 2, w], f32, tag="lsb", name="lsb")
            nc.vector.tensor_copy(out=lsb, in_=oAB[HD:HD + 1, :, csl])
            lr = small_pool.tile([1, 2, w], f32, tag="lr", name="lr")
            with nc.allow_low_precision(reason="recip"):
                nc.vector.reciprocal_approx_fast(out=lr, in_=lsb)
            Rsb = small_pool.tile([HD, 2, w], f32, tag="Rsb", name="Rsb")
            nc.gpsimd.partition_broadcast(Rsb, lr)
            nc.vector.tensor_mul(AT[p][0:HD, qsl], oAB[0:HD, 0, csl],
                                 Rsb[:, 0, :])
            nc.vector.tensor_mul(AT[p][HD:128, qsl], oAB[0:HD, 1, csl],
                                 Rsb[:, 1, :])

        def outproj_piece(j, rt):
            # output projection for one 128-row tile (needs both pairs' AT)
            rsl = slice(j * QCH + rt * 128, j * QCH + (rt + 1) * 128)
            po2 = psum_s.tile([128, 2, 512], f32, tag="s2", name="po2")
            for nch in range(DIM // 512):
                for pp in range(PAIRS):
                    nc.tensor.matmul(
                        po2[:, nch, :], AT[pp][:, rsl],
                        ow_sb[:, pp, nch * 512:(nch + 1) * 512],
                        start=(pp == 0), stop=(pp == PAIRS - 1))
            ot = outst_pool.tile([128, 2, 512], bf16, tag="ot", name="ot")
            nc.vector.tensor_copy(out=ot[:, 0, :], in_=po2[:, 0, :])
            nc.sync.dma_start(out=out[rsl, 0:512], in_=ot[:, 0, :])
            nc.scalar.copy(out=ot[:, 1, :], in_=po2[:, 1, :])
            nc.sync.dma_start(out=out[rsl, 512:1024], in_=ot[:, 1, :])

        pending_norm = None
        proj_q = []
        feed = []
        for j in range(S // QCH):
            if j + 1 < S // QCH:
                feed = round_feed(j + 1)
            if j == 0:
                feed = feed0 + feed
            for p in range(PAIRS):
                nkt = 4 * (j + 1)
                # norm of the previous chunk is pure DVE/ACT work; emit it
                # before this chunk's matmuls UNLESS this chunk's causal masks
                # start immediately (j == 0), where it would delay them.
                if pending_norm is not None and j != 0:
                    finalize_norm(*pending_norm)
                    pending_norm = None
                all_diag = (j == 0)
                oAB = psum_o.tile([HD + 1, 2, QCH], f32, tag="o", name="oAB")
                # 1-deep software pipeline: PV(kt) is emitted after S(kt+1) so
                # the PE never sits at a PV waiting for its exp.
                pv_prev = None
                for kt in range(nkt):
                    ksl = slice(kt * 128, (kt + 1) * 128)
                    di = kt - 4 * j
                    qs = 128 * di if di > 0 else 0  # live q range start
                    gsl = slice(j * QCH + qs, (j + 1) * QCH)
                    sAB = psum_s.tile([128, 2, QCH], f32, tag="s2", name="sAB")
                    for hh in range(2):
                        nc.tensor.matmul(sAB[:, hh, qs:QCH],
                                         KT[p][hh * 64:(hh + 1) * 64, ksl],
                                         QT[p][hh * 64:(hh + 1) * 64, gsl],
                                         start=True, stop=True)
                    pAB = pt_pool.tile([128, 2, QCH], bf16, tag="pt", name="pAB")
                    nc.scalar.activation(out=pAB[:, :, qs:QCH],
                                         in_=sAB[:, :, qs:QCH], func=Exp)
                    if di >= 0:  # diagonal sub-block: apply causal mask
                        m2 = bass.AP(tensor=mask_sb.tensor, offset=mask_sb.offset,
                                     ap=[list(mask_sb.ap[0]), [0, 2],
                                         list(mask_sb.ap[1])])
                        nc.vector.tensor_mul(pAB[:, :, qs:qs + 128],
                                             pAB[:, :, qs:qs + 128], m2)
                    if pv_prev is not None:
                        kprev, pprev, qsp = pv_prev
                        for hh in range(2):
                            nc.tensor.matmul(oAB[:, hh, qsp:QCH],
                                             Vaug[:, kprev, 2 * p + hh, :],
                                             pprev[:, hh, qsp:QCH],
                                             start=(kprev == 0), stop=False)
                    pv_prev = (kt, pAB, qs)
                    if feed:
                        feed.pop(0)()
                    if (pending_norm is not None and not all_diag
                            and kt == min(3, nkt - 1)):
                        finalize_norm(*pending_norm)
                        pending_norm = None
                    if (j == S // QCH - 1 and kt % 2 == 1 and proj_q
                            and (p == 0 or kt >= nkt - 8)):
                        outproj_piece(*proj_q.pop(0))
                kprev, pprev, qsp = pv_prev
                for hh in range(2):
                    nc.tensor.matmul(oAB[:, hh, qsp:QCH],
                                     Vaug[:, kprev, 2 * p + hh, :],
                                     pprev[:, hh, qsp:QCH],
                                     start=(kprev == 0), stop=True)
                if pending_norm is not None:
                    # all-diagonal chunk: its masks are done now, safe to emit
                    finalize_norm(*pending_norm)
                pending_norm = (p, j, oAB)
                if p == PAIRS - 1 and j < S // QCH - 1:
                    proj_q += [(j, rt) for rt in range(QCH // 128)]
            while feed:
                feed.pop(0)()
        while proj_q:
            outproj_piece(*proj_q.pop(0))
        fp, fj, foAB = pending_norm
        finalize_norm(fp, fj, foAB, 0, 128)
        # keep the PE clock ramped across the finalize chain's ~3us PE gap:
        # the 16 output-projection matmuls after it otherwise run at the
        # dropped 1.2 GHz p-state (634ns vs 215ns per 512-row stream)
        for _ in range(6):
            wps = psum_s.tile([128, QCH], f32, tag="s2", name="wps")
            nc.tensor.matmul(wps, zw, zx, start=True, stop=True)
        for rt in range(1, 4):
            finalize_norm(fp, fj, foAB, rt * 128, (rt + 1) * 128)
            outproj_piece(fj, rt - 1)
            for _ in range(3):
                wps = psum_s.tile([128, QCH], f32, tag="s2", name="wps")
                nc.tensor.matmul(wps, zw, zx, start=True, stop=True)
        outproj_piece(fj, 3)

    nc.compile()
    return nc


def _get_nc():
    global _nc_cache
    if _nc_cache is None:
        _nc_cache = _build()
    return _nc_cache


def _prep_inputs(x, qw, kw, vw, ow):
    import ml_dtypes
    bf16 = ml_dtypes.bfloat16

    # undo interleaved head split: head h = cols h::16 -> contiguous blocks
    perm = np.concatenate([np.arange(h, DIM, HEADS) for h in range(HEADS)])
    qw_p = (qw[:, perm] / np.float32(np.sqrt(DIM))).astype(np.float32)
    kw_p = kw[:, perm]
    vw_p = vw[:, perm]
    ow_p = ow[perm, :]

    kp = np.arange(128)[:, None]
    qf = np.arange(128)[None, :]
    mask = (kp <= qf).astype(bf16)

    def pack_w(w):  # [1024, 256] -> [128, 8, 256] (kt-major per partition)
        return np.ascontiguousarray(
            w.reshape(8, 128, 256).transpose(1, 0, 2)).astype(bf16)

    def pack_w_ct(w):  # [1024, 256] -> [128, 2ct, 8kt, 128]
        return np.ascontiguousarray(
            w.reshape(8, 128, 2, 128).transpose(1, 2, 0, 3)).astype(bf16)

    in_maps = []
    for c in range(N_CORES):
        b, hg = c // 4, c % 4
        csl = slice(hg * 256, (hg + 1) * 256)
        # x^T packed [128 p, 4 j, 8 kt, 512 n] -> [128, 32, 512]
        xt = np.ascontiguousarray(
            x[b].T.reshape(8, 128, 4, QCH).transpose(1, 2, 0, 3)
            .reshape(128, 32, QCH)).astype(bf16)
        owc = np.ascontiguousarray(
            ow_p[csl, :].reshape(2, 128, DIM).transpose(1, 0, 2)).astype(bf16)
        in_maps.append({
            "xT": xt,
            "qw": pack_w_ct(qw_p[:, csl]),
            "kw": pack_w_ct(kw_p[:, csl]),
            "vw": pack_w(vw_p[:, csl]),
            "ow": owc,
            "mask": mask,
        })
    return in_maps


def kernel(x, qw, kw, vw, ow, _trace=False):
    from concourse.bass_utils import run_bass_kernel_spmd

    if _trace:
        _install_ntff_hook()

    nc = _get_nc()
    in_maps = _prep_inputs(x, qw, kw, vw, ow)
    res = run_bass_kernel_spmd(nc, in_maps, core_ids=list(range(N_CORES)),
                               trace=_trace)
    parts = [np.asarray(r["out"], dtype=np.float32) for r in res.results]
    outb = [parts[0] + parts[1] + parts[2] + parts[3],
            parts[4] + parts[5] + parts[6] + parts[7]]
    full = np.stack(outb).astype(np.float32)
    if _trace:
        kernel.last_results = res
        if res.exec_time_ns is not None:
            print(f"HW exec time: {res.exec_time_ns} ns")
        if res.instructions_and_trace:
            print(f"trace: {res.instructions_and_trace[1]}")
    return full


def _install_ntff_hook():
    """The image's antenv lacks axon_hooks; synthesize it so trace=True works."""
    import types
    if 'antenv.axon_hooks' in sys.modules:
        return
    mod = types.ModuleType('antenv.axon_hooks')
    mod._hook = None
    mod.set_axon_ntff_profile_hook = lambda h: setattr(mod, '_hook', h)
    mod.get_axon_ntff_profile_hook = lambda: mod._hook
    sys.modules['antenv.axon_hooks'] = mod
    import antenv
    antenv.axon_hooks = mod
    from trn_agent_boot.trn_boot import _ntff_profile_via_ctypes
    mod.set_axon_ntff_profile_hook(
        _ntff_profile_via_ctypes('/opt/axon/libaxon_pjrt.so'))
